# revision 5
# baseline (speedup 1.0000x reference)
"""Trainium2 Bass kernel for nn_AudioVisualInteractionGraph.

Sharding: data-parallel over batch B=8, one batch element per NeuronCore.
Per core: vm = V @ Wv, am = A @ Wa (PE fp32 via PE-transposed operands);
sq[s,t] = |vm_t|^2 + |am_s|^2 - 2 vm_t.am_s (PE + broadcast add);
l2 = sqrt(sq), y = exp(-l2) on ACT (bit-exact vs device XLA incl. denormals);
stable top-8 per audio row via DVE max8 + max_index on float32(y bits)
("integerized" ordering: exact for denormal/zero ties, order-preserving for
normals). Host: counts-based gather-mean (tiny).
"""
import numpy as np

import concourse.bass as bass
import concourse.bacc as bacc
import concourse.mybir as mybir
from concourse.tile import TileContext
from concourse.bass_utils import run_bass_kernel_spmd

AFT = mybir.ActivationFunctionType
DT = mybir.dt

B, T, D = 8, 2048, 256
NB = T // 128          # 16 row-blocks
NCH = D // 128         # 2 contraction chunks
NN = T // 512          # 4 N-chunks of 512


def _build():
    nc = bacc.Bacc()
    v_in = nc.declare_dram_parameter("v", [T, D], DT.float32, isOutput=False)
    a_in = nc.declare_dram_parameter("a", [T, D], DT.float32, isOutput=False)
    wv_in = nc.declare_dram_parameter("wv", [D, D], DT.float32, isOutput=False)
    wa_in = nc.declare_dram_parameter("wa", [D, D], DT.float32, isOutput=False)
    eye_in = nc.declare_dram_parameter("eye", [128, 128], DT.float32, isOutput=False)
    ones_in = nc.declare_dram_parameter("ones", [128, 1], DT.float32, isOutput=False)
    idx_out = nc.declare_dram_parameter("idx8", [T, 8], DT.uint32, isOutput=True)

    with TileContext(nc) as tc:
        with (
            tc.tile_pool(name="w", bufs=1) as wp,
            tc.tile_pool(name="xt", bufs=1) as xtp,
            tc.tile_pool(name="proj", bufs=1) as pp,
            tc.tile_pool(name="ld", bufs=3) as ldp,
            tc.tile_pool(name="blk", bufs=2) as bp,
            tc.tile_pool(name="ps", bufs=2, space="PSUM") as ps,
            tc.tile_pool(name="ps2", bufs=1, space="PSUM") as ps2,
        ):
            eye = wp.tile([128, 128], DT.float32, tag="eye")
            nc.sync.dma_start(eye[:], eye_in[:])
            ones = wp.tile([128, 1], DT.float32, tag="ones")
            nc.sync.dma_start(ones[:], ones_in[:])
            onesrow = wp.tile([1, 128], DT.float32, tag="onesrow")
            nc.sync.dma_start(onesrow[:], ones_in[:].rearrange("a b -> b a"))
            wv = [wp.tile([128, D], DT.float32, tag=f"wv{k}", name=f"wv{k}") for k in range(NCH)]
            wa = [wp.tile([128, D], DT.float32, tag=f"wa{k}", name=f"wa{k}") for k in range(NCH)]
            for k in range(NCH):
                nc.sync.dma_start(wv[k][:], wv_in[128 * k:128 * (k + 1), :])
                nc.sync.dma_start(wa[k][:], wa_in[128 * k:128 * (k + 1), :])

            # ---- stage 1: transpose V, A -> VT, AT  [d-chunk][128, 2048]
            vt = [xtp.tile([128, T], DT.float32, tag=f"vt{j}", name=f"vt{j}") for j in range(NCH)]
            at = [xtp.tile([128, T], DT.float32, tag=f"at{j}", name=f"at{j}") for j in range(NCH)]
            for src, dst, oneng in ((v_in, vt, True), (a_in, at, False)):
                for tb in range(NB):
                    xt_in = ldp.tile([128, D], DT.float32, tag="xld")
                    nc.sync.dma_start(xt_in[:], src[128 * tb:128 * (tb + 1), :])
                    for j in range(NCH):
                        pt = ps.tile([128, 512], DT.float32, tag="scratch")
                        nc.tensor.transpose(
                            pt[:, 0:128], xt_in[:, 128 * j:128 * (j + 1)], eye[:])
                        dstap = dst[j][:, 128 * tb:128 * (tb + 1)]
                        if oneng:
                            nc.vector.tensor_copy(dstap, pt[:, 0:128])
                        else:
                            nc.scalar.copy(dstap, pt[:, 0:128])

            # ---- stage 2: projections vmT = Wv^T V^T (unscaled), amTs = -2 Wa^T A^T
            vmt = [pp.tile([128, T], DT.float32, tag=f"vmt{j}", name=f"vmt{j}") for j in range(NCH)]
            amts = [pp.tile([128, T], DT.float32, tag=f"amts{j}", name=f"amts{j}") for j in range(NCH)]
            for j in range(NCH):
                for n in range(NN):
                    pv = ps.tile([128, 512], DT.float32, tag="scratch")
                    for k in range(NCH):
                        nc.tensor.matmul(
                            pv[:], wv[k][:, 128 * j:128 * (j + 1)],
                            vt[k][:, 512 * n:512 * (n + 1)],
                            start=(k == 0), stop=(k == NCH - 1))
                    nc.vector.tensor_copy(vmt[j][:, 512 * n:512 * (n + 1)], pv[:])
                    pa = ps.tile([128, 512], DT.float32, tag="scratch")
                    for k in range(NCH):
                        nc.tensor.matmul(
                            pa[:], wa[k][:, 128 * j:128 * (j + 1)],
                            at[k][:, 512 * n:512 * (n + 1)],
                            start=(k == 0), stop=(k == NCH - 1))
                    nc.scalar.mul(amts[j][:, 512 * n:512 * (n + 1)], pa[:], -2.0)

            # ---- stage 3: norms. nv[t] = sum_d vmT^2, na[s] = sum amTs^2 / 4
            nv = pp.tile([1, T], DT.float32, tag="nv")
            na = pp.tile([1, T], DT.float32, tag="na")
            tmp = pp.tile([128, T], DT.float32, tag="sqtmp")
            pnrm = [ps2.tile([128, 512], DT.float32, tag=f"pss{n}", name=f"pnrm{n}") for n in range(NN)]
            for k in range(NCH):
                nc.vector.tensor_mul(tmp[:], vmt[k][:], vmt[k][:])
                for n in range(NN):
                    nc.tensor.matmul(pnrm[n][0:1, :], ones[:],
                                     tmp[:, 512 * n:512 * (n + 1)],
                                     start=(k == 0), stop=(k == NCH - 1))
            for n in range(NN):
                nc.vector.tensor_copy(nv[:, 512 * n:512 * (n + 1)], pnrm[n][0:1, :])
            pnrm2 = [ps2.tile([128, 512], DT.float32, tag=f"pss{n}", name=f"pnrm2_{n}") for n in range(NN)]
            for k in range(NCH):
                nc.vector.tensor_mul(tmp[:], amts[k][:], amts[k][:])
                for n in range(NN):
                    nc.tensor.matmul(pnrm2[n][0:1, :], ones[:],
                                     tmp[:, 512 * n:512 * (n + 1)],
                                     start=(k == 0), stop=(k == NCH - 1))
            for n in range(NN):
                nc.vector.tensor_copy(na[:, 512 * n:512 * (n + 1)], pnrm2[n][0:1, :])

            # nvb[128, T]: broadcast nv down partitions (PE outer product K=1)
            nvb = pp.tile([128, T], DT.float32, tag="nvb")
            for n in range(NN):
                pb = ps.tile([128, 512], DT.float32, tag="scratch")
                nc.tensor.matmul(pb[:], onesrow[:], nv[:, 512 * n:512 * (n + 1)],
                                 start=True, stop=True)
                nc.scalar.copy(nvb[:, 512 * n:512 * (n + 1)], pb[:])

            # naT[128, NB]: na transposed chunkwise (K=1 matmul with scalar one),
            # scaled by 0.25 (undo the -2 on amTs)
            nat = pp.tile([128, NB], DT.float32, tag="nat")
            for sb in range(NB):
                pn = ps.tile([128, 512], DT.float32, tag="scratch")
                nc.tensor.matmul(pn[:, 0:1], na[:, 128 * sb:128 * (sb + 1)],
                                 onesrow[0:1, 0:1], start=True, stop=True)
                nc.scalar.mul(nat[:, sb:sb + 1], pn[:, 0:1], 0.25)

            # ---- stage 4: per s-block distances + sqrt + exp + stable top-8
            for sb in range(NB):
                pss = [ps2.tile([128, 512], DT.float32, tag=f"pss{n}",
                                name=f"pss{sb}_{n}") for n in range(NN)]
                for n in range(NN):
                    for k in range(NCH):
                        nc.tensor.matmul(
                            pss[n][:], amts[k][:, 128 * sb:128 * (sb + 1)],
                            vmt[k][:, 512 * n:512 * (n + 1)],
                            start=(k == 0), stop=(k == NCH - 1))
                u = bp.tile([128, T], DT.float32, tag="u")
                for n in range(NN):
                    nc.vector.tensor_add(u[:, 512 * n:512 * (n + 1)], pss[n][:],
                                         nvb[:, 512 * n:512 * (n + 1)])
                l2 = bp.tile([128, T], DT.float32, tag="l2")
                nc.scalar.activation(l2[:], u[:], AFT.Sqrt, bias=nat[:, sb:sb + 1])
                y = bp.tile([128, T], DT.float32, tag="y")
                nc.scalar.activation(y[:], l2[:], AFT.Exp, scale=-1.0)
                ycvt = bp.tile([128, T], DT.float32, tag="ycvt")
                nc.vector.tensor_copy(ycvt[:], y[:].bitcast(DT.uint32))
                v8 = bp.tile([128, 8], DT.float32, tag="v8")
                nc.vector.max(v8[:], ycvt[:])
                i8 = bp.tile([128, 8], DT.uint32, tag="i8")
                nc.vector.max_index(i8[:], v8[:], ycvt[:])
                nc.sync.dma_start(idx_out[128 * sb:128 * (sb + 1), :], i8[:])
    nc.finalize()
    return nc


_NC_CACHE = {}


def _get_nc():
    if "nc" not in _NC_CACHE:
        _NC_CACHE["nc"] = _build()
    return _NC_CACHE["nc"]


def kernel(visual_features, audio_features, visual_weights, audio_weights,
           num_neighbors):
    V = np.ascontiguousarray(np.asarray(visual_features, dtype=np.float32))
    A = np.ascontiguousarray(np.asarray(audio_features, dtype=np.float32))
    Wv = np.ascontiguousarray(np.asarray(visual_weights, dtype=np.float32))
    Wa = np.ascontiguousarray(np.asarray(audio_weights, dtype=np.float32))
    k = int(num_neighbors)
    assert V.shape == (B, T, D) and A.shape == (B, T, D)
    assert 1 <= k <= 8, f"num_neighbors={k} unsupported"

    eye = np.eye(128, dtype=np.float32)
    ones = np.ones((128, 1), np.float32)
    nc = _get_nc()
    in_maps = [{"v": V[b], "a": A[b], "wv": Wv, "wa": Wa, "eye": eye, "ones": ones}
               for b in range(B)]
    res = run_bass_kernel_spmd(nc, in_maps, list(range(B))).results

    out1 = np.zeros((B, k, D), np.float64)
    out2 = np.zeros((B, k, D), np.float64)
    for b in range(B):
        idx = res[b]["idx8"]            # [T(s), 8] uint32
        for j in range(k):
            c = np.bincount(idx[:, j].astype(np.int64), minlength=T).astype(np.float64)
            out1[b, j] = c @ V[b].astype(np.float64) / T
            out2[b, j] = c @ A[b].astype(np.float64) / T
    return out1.astype(np.float32), out2.astype(np.float32)
